# revision 3
# baseline (speedup 1.0000x reference)
"""Trainium2 Bass kernel for BioREDirect-style ragged attention pooling + heads.

Computation (per batch b, per group g in {rel, nov, dir}):
    G      = seq[b, idx_g[b, :], :]                       # [K=64, H=768] gather
    s      = G @ G[0]  (masked to len_g[b], softmax)      # ragged row-0 attention
    pooled = softmax(s) @ G   (fallback seq[b,0] if len==0)
    logits = tanh(pooled @ Wd_g.T + bd_g) @ Wc_g.T + bc_g

Sharding: pure data-parallel over batch (B=64 -> 8 per core), weights replicated.

Device-side design per (g, b) pair (24 per core):
  - indirect-DMA gather of 65 rows (64 indexed + seq[b,0] appended as a 65th
    attention candidate; host-built masks make it win exactly when len==0)
  - query broadcast via PE ones-matmul, scores via one fused DVE
    tensor_tensor_reduce (multiply + free-axis sum) -> scores [65,1]
  - tiny PE transposes shuttle [65,1]<->[1,65] for the masked softmax
    (exp with accum_out gives the denominator for free)
  - pooled = a^T G on PE; pooled rows DMA'd into a per-group [8, 768] tile
Head per group: PE chunk-transposes of pooled reps, dense matmul with
Wd^T chunks as stationary weights (output feature-major so tanh bias is a
native per-partition ACT bias), classifier + bias via rank-1 PSUM accumulate.
"""

import numpy as np

import concourse.bass as bass
import concourse.tile as tile
from concourse import bacc, mybir
from concourse.bass import ts

F32 = mybir.dt.float32
I32 = mybir.dt.int32
AX = mybir.AluOpType
ACT_FN = mybir.ActivationFunctionType

# Problem shapes (hardcoded per contest contract)
B, S, H, K = 64, 512, 768, 64
N_CORES = 8
BC = B // N_CORES          # batches per core
KP = K + 1                 # 64 gathered rows + fallback row seq[b, 0]
NG = 3                     # rel, nov, dir
NPAIR = NG * BC            # 24 (g, b) pairs per core
CH = H // 128              # 6 chunks of 128 along H
GROUPS = ("rel", "nov", "dir")
NLAB = (9, 3, 3)
NEG = -1.0e30


def _build_program():
    nc = bacc.Bacc("TRN2", debug=False, num_devices=N_CORES)

    seq = nc.dram_tensor("seq", [BC * S, H], F32, kind="ExternalInput")
    gidx = nc.dram_tensor("gidx", [KP, NPAIR], I32, kind="ExternalInput")
    maskT = nc.dram_tensor("maskT", [KP, NPAIR], F32, kind="ExternalInput")
    wdT, bd, wcT, bc, outs = [], [], [], [], []
    for g, nl in zip(GROUPS, NLAB):
        wdT.append(nc.dram_tensor(f"wdT_{g}", [H, H], F32, kind="ExternalInput"))
        bd.append(nc.dram_tensor(f"bd_{g}", [H], F32, kind="ExternalInput"))
        wcT.append(nc.dram_tensor(f"wcT_{g}", [H, nl], F32, kind="ExternalInput"))
        bc.append(nc.dram_tensor(f"bc_{g}", [1, nl], F32, kind="ExternalInput"))
        outs.append(nc.dram_tensor(f"out_{g}", [BC, nl], F32, kind="ExternalOutput"))

    from concourse.masks import make_identity

    with tile.TileContext(nc) as tc:
        with tc.tile_pool(name="const", bufs=1) as cpool:
            ident = cpool.tile([128, 128], F32)
            make_identity(nc, ident[:])
            ones65 = cpool.tile([1, KP], F32)
            nc.gpsimd.memset(ones65[:], 1.0)

            gidx_s = cpool.tile([KP, NPAIR], I32)
            nc.sync.dma_start(out=gidx_s[:], in_=gidx.ap())
            maskT_s = cpool.tile([KP, NPAIR], F32)
            nc.sync.dma_start(out=maskT_s[:], in_=maskT.ap())

            wdT_s, bdT_s, wcT_s, bc_s, prows = [], [], [], [], []
            for g in range(NG):
                w = cpool.tile([128, CH * H], F32, tag=f"wdT{g}")
                nc.sync.dma_start(
                    out=w[:].rearrange("p (c i) -> p c i", i=H),
                    in_=wdT[g].ap().rearrange("(c p) i -> p c i", p=128),
                )
                wdT_s.append(w)
                bt = cpool.tile([128, CH], F32, tag=f"bdT{g}")
                nc.sync.dma_start(
                    out=bt[:], in_=bd[g].ap().rearrange("(c p) -> p c", p=128)
                )
                bdT_s.append(bt)
                wc = cpool.tile([128, CH * NLAB[g]], F32, tag=f"wcT{g}")
                nc.sync.dma_start(
                    out=wc[:].rearrange("p (c l) -> p c l", l=NLAB[g]),
                    in_=wcT[g].ap().rearrange("(c p) l -> p c l", p=128),
                )
                wcT_s.append(wc)
                bcs = cpool.tile([1, NLAB[g]], F32, tag=f"bc{g}")
                nc.sync.dma_start(out=bcs[:], in_=bc[g].ap())
                bc_s.append(bcs)
                pr = cpool.tile([BC, H], F32, tag=f"prows{g}")
                prows.append(pr)

            # ---- phase 1: 24 ragged attention-pooling pairs ----
            with (
                tc.tile_pool(name="gat", bufs=3) as gpool,
                tc.tile_pool(name="prod", bufs=2) as prodpool,
                tc.tile_pool(name="small", bufs=4) as spool,
                tc.tile_pool(name="row", bufs=4) as rowpool,
                tc.tile_pool(name="prow", bufs=3) as prowpool,
                tc.tile_pool(name="qb_ps", bufs=2, space="PSUM") as qbpool,
                tc.tile_pool(name="pool_ps", bufs=1, space="PSUM") as plpool,
                tc.tile_pool(name="sm_ps", bufs=2, space="PSUM") as smpool,
            ):
                for p in range(NPAIR):
                    g, b = divmod(p, BC)
                    G = gpool.tile([KP, H], F32, tag="G")
                    nc.gpsimd.indirect_dma_start(
                        out=G[:],
                        out_offset=None,
                        in_=seq.ap(),
                        in_offset=bass.IndirectOffsetOnAxis(
                            ap=gidx_s[:, p : p + 1], axis=0
                        ),
                    )
                    # query broadcast (qb[k, :] = G[0, :]) in two half-bank
                    # tiles; scores[k] = sum_h G[k,h]*qb[k,h] via fused
                    # multiply + free-axis accumulate (scalar_tensor_tensor;
                    # tensor_tensor_reduce crashes this runtime's DVE)
                    HH = H // 2
                    halves = []
                    for half in range(2):
                        qb = qbpool.tile([KP, HH], F32, tag="qb")
                        lo = half * HH
                        nc.tensor.matmul(
                            qb[:], lhsT=ones65[:], rhs=G[0:1, lo : lo + HH],
                            start=True, stop=True,
                        )
                        prod = prodpool.tile([KP, HH], F32, tag="prod")
                        acc = spool.tile([KP, 1], F32, tag=f"sacc{half}")
                        nc.vector.scalar_tensor_tensor(
                            out=prod[:], in0=G[:, lo : lo + HH], scalar=1.0,
                            in1=qb[:], op0=AX.mult, op1=AX.mult,
                            accum_out=acc[:],
                        )
                        halves.append(acc)
                    sT = spool.tile([KP, 1], F32, tag="sT")
                    nc.vector.tensor_add(out=sT[:], in0=halves[0][:], in1=halves[1][:])
                    smT = spool.tile([KP, 1], F32, tag="smT")
                    nc.vector.tensor_tensor(
                        out=smT[:], in0=sT[:], in1=maskT_s[:, p : p + 1], op=AX.add
                    )
                    srow = smpool.tile([1, KP], F32, tag="srow")
                    nc.tensor.transpose(
                        out=srow[:], in_=smT[:], identity=ident[:KP, :KP]
                    )
                    mx = spool.tile([1, 1], F32, tag="mx")
                    nc.vector.reduce_max(out=mx[:], in_=srow[:], axis=mybir.AxisListType.X)
                    nmx = spool.tile([1, 1], F32, tag="nmx")
                    nc.scalar.mul(nmx[:], mx[:], -1.0)
                    erow = rowpool.tile([1, KP], F32, tag="erow")
                    den = spool.tile([1, 1], F32, tag="den")
                    nc.scalar.activation(
                        out=erow[:], in_=srow[:], func=ACT_FN.Exp,
                        bias=nmx[:], scale=1.0, accum_out=den[:],
                    )
                    rden = spool.tile([1, 1], F32, tag="rden")
                    nc.vector.reciprocal(rden[:], den[:])
                    arow = rowpool.tile([1, KP], F32, tag="arow")
                    nc.scalar.mul(arow[:], erow[:], rden[:])
                    aTp = smpool.tile([KP, 1], F32, tag="aTp")
                    nc.tensor.transpose(
                        out=aTp[:], in_=arow[:], identity=ident[:1, :1]
                    )
                    aT = spool.tile([KP, 1], F32, tag="aT")
                    nc.vector.tensor_copy(out=aT[:], in_=aTp[:])
                    pooled = plpool.tile([1, H], F32, tag="pooled")
                    nc.tensor.matmul(
                        pooled[:, 0:512], lhsT=aT[:], rhs=G[:, 0:512],
                        start=True, stop=True,
                    )
                    nc.tensor.matmul(
                        pooled[:, 512:H], lhsT=aT[:], rhs=G[:, 512:H],
                        start=True, stop=True,
                    )
                    prow = prowpool.tile([1, H], F32, tag="prow")
                    if p % 2 == 0:
                        nc.scalar.copy(prow[:], pooled[:])
                    else:
                        nc.vector.tensor_copy(out=prow[:], in_=pooled[:])
                    nc.sync.dma_start(out=prows[g][b : b + 1, :], in_=prow[:])

            # ---- phase 2: pooler dense + tanh + classifier per group ----
            with (
                tc.tile_pool(name="head", bufs=1) as hpool,
                tc.tile_pool(name="tp_ps", bufs=2, space="PSUM") as tppool,
                tc.tile_pool(name="h_ps", bufs=2, space="PSUM") as hpspool,
                tc.tile_pool(name="lg_ps", bufs=2, space="PSUM") as lgpool,
            ):
                for g in range(NG):
                    nl = NLAB[g]
                    wdv = wdT_s[g][:].rearrange("p (c i) -> p c i", i=H)
                    wcv = wcT_s[g][:].rearrange("p (c l) -> p c l", l=nl)
                    ptT = hpool.tile([128, CH * BC], F32, tag=f"ptT{g}")
                    for c in range(CH):
                        tp = tppool.tile([128, BC], F32, tag="tp")
                        nc.tensor.transpose(
                            out=tp[:], in_=prows[g][:, ts(c, 128)],
                            identity=ident[:BC, :BC],
                        )
                        nc.vector.tensor_copy(out=ptT[:, ts(c, BC)], in_=tp[:])
                    hT = hpool.tile([128, CH * BC], F32, tag=f"hT{g}")
                    for ic in range(CH):
                        hps = hpspool.tile([128, BC], F32, tag="hps")
                        for hc in range(CH):
                            nc.tensor.matmul(
                                hps[:],
                                lhsT=wdv[:, hc, ts(ic, 128)],
                                rhs=ptT[:, ts(hc, BC)],
                                start=(hc == 0), stop=(hc == CH - 1),
                            )
                        nc.scalar.activation(
                            out=hT[:, ts(ic, BC)], in_=hps[:], func=ACT_FN.Tanh,
                            bias=bdT_s[g][:, ic : ic + 1], scale=1.0,
                        )
                    lg = lgpool.tile([BC, nl], F32, tag="lg")
                    for ic in range(CH):
                        nc.tensor.matmul(
                            lg[:], lhsT=hT[:, ts(ic, BC)], rhs=wcv[:, ic, :],
                            start=(ic == 0), stop=False,
                        )
                    nc.tensor.matmul(
                        lg[:], lhsT=ones65[0:1, 0:BC], rhs=bc_s[g][:],
                        start=False, stop=True,
                    )
                    lgs = hpool.tile([BC, nl], F32, tag=f"lgs{g}")
                    nc.vector.tensor_copy(out=lgs[:], in_=lg[:])
                    nc.sync.dma_start(out=outs[g].ap(), in_=lgs[:])

    nc.compile()
    return nc


_STATE = None


def _get_nc():
    global _STATE
    if _STATE is None:
        _STATE = _build_program()
    return _STATE


def _prep_in_maps(inputs):
    seq = np.ascontiguousarray(np.asarray(inputs["sequence_output"], dtype=np.float32))
    idxs = {g: np.asarray(inputs[f"{g}_idx"]).astype(np.int64) for g in GROUPS}
    lens = {g: np.asarray(inputs[f"{g}_len"]).astype(np.int64) for g in GROUPS}

    shared = {}
    for g, nl in zip(GROUPS, NLAB):
        wd = np.asarray(inputs[f"{g}_dense_W"], dtype=np.float32)
        wc = np.asarray(inputs[f"{g}_cls_W"], dtype=np.float32)
        shared[f"wdT_{g}"] = np.ascontiguousarray(wd.T)
        shared[f"bd_{g}"] = np.asarray(inputs[f"{g}_dense_b"], np.float32).reshape(H)
        shared[f"wcT_{g}"] = np.ascontiguousarray(wc.T)
        shared[f"bc_{g}"] = np.asarray(inputs[f"{g}_cls_b"], np.float32).reshape(1, nl)

    in_maps = []
    ar64 = np.arange(K)
    for c in range(N_CORES):
        b0 = c * BC
        gidx = np.zeros((KP, NPAIR), np.int32)
        maskT = np.zeros((KP, NPAIR), np.float32)
        for g, gname in enumerate(GROUPS):
            idx = idxs[gname][b0 : b0 + BC]          # [BC, K]
            ln = lens[gname][b0 : b0 + BC]           # [BC]
            cols = slice(g * BC, (g + 1) * BC)
            gidx[:K, cols] = (idx + (np.arange(BC) * S)[:, None]).T
            gidx[K, cols] = np.arange(BC) * S
            maskT[:K, cols] = np.where(ar64[:, None] < ln[None, :], 0.0, NEG)
            maskT[K, cols] = np.where(ln == 0, 0.0, NEG)
        in_maps.append(
            {
                "seq": seq[b0 : b0 + BC].reshape(BC * S, H),
                "gidx": gidx,
                "maskT": maskT,
                **shared,
            }
        )
    return in_maps


_EXEC_CACHE = None


def _get_exec():
    """Cached jitted shard_map executable (mirrors bass2jax.run_bass_via_pjrt)."""
    global _EXEC_CACHE
    if _EXEC_CACHE is not None:
        return _EXEC_CACHE
    import jax
    from jax.sharding import Mesh, PartitionSpec
    from jax.experimental.shard_map import shard_map
    from concourse import bass2jax, mybir as mb

    nc = _get_nc()
    bass2jax.install_neuronx_cc_hook()
    partition_name = nc.partition_id_tensor.name if nc.partition_id_tensor else None

    in_names, out_names, out_avals, zero_outs = [], [], [], []
    for alloc in nc.m.functions[0].allocations:
        if not isinstance(alloc, mb.MemoryLocationSet):
            continue
        name = alloc.memorylocations[0].name
        if alloc.kind == "ExternalInput":
            if name != partition_name:
                in_names.append(name)
        elif alloc.kind == "ExternalOutput":
            shape = tuple(alloc.tensor_shape)
            dtype = mb.dt.np(alloc.dtype)
            out_names.append(name)
            out_avals.append(jax.core.ShapedArray(shape, dtype))
            zero_outs.append(np.zeros(shape, dtype))
    n_params = len(in_names)
    n_outs = len(out_avals)
    all_in_names = list(in_names) + list(out_names)
    if partition_name is not None:
        all_in_names.append(partition_name)
    donate = tuple(range(n_params, n_params + n_outs))

    def _body(*args):
        operands = list(args)
        if partition_name is not None:
            operands.append(bass2jax.partition_id_tensor())
        return tuple(
            bass2jax._bass_exec_p.bind(
                *operands,
                out_avals=tuple(out_avals),
                in_names=tuple(all_in_names),
                out_names=tuple(out_names),
                lowering_input_output_aliases=(),
                sim_require_finite=True,
                sim_require_nnan=True,
                nc=nc,
            )
        )

    devices = jax.devices()[:N_CORES]
    mesh = Mesh(np.asarray(devices), ("core",))
    sharded = jax.jit(
        shard_map(
            _body,
            mesh=mesh,
            in_specs=(PartitionSpec("core"),) * (n_params + n_outs),
            out_specs=(PartitionSpec("core"),) * n_outs,
            check_rep=False,
        ),
        donate_argnums=donate,
        keep_unused=True,
    )
    _EXEC_CACHE = (sharded, in_names, out_names, out_avals, zero_outs, mesh)
    return _EXEC_CACHE


def _run(in_maps):
    import numpy as _np

    sharded, in_names, out_names, out_avals, zero_outs, _ = _get_exec()
    concat_in = [
        _np.concatenate([_np.asarray(in_maps[c][n]) for c in range(N_CORES)], axis=0)
        for n in in_names
    ]
    concat_zeros = [
        _np.zeros((N_CORES * z.shape[0], *z.shape[1:]), z.dtype) for z in zero_outs
    ]
    out_arrs = sharded(*concat_in, *concat_zeros)
    return {
        name: _np.asarray(out_arrs[i]).reshape(N_CORES, *out_avals[i].shape)
        for i, name in enumerate(out_names)
    }


def _assemble(per_core):
    res = []
    for g in GROUPS:
        res.append(np.ascontiguousarray(per_core[f"out_{g}"].reshape(B, -1)))
    return tuple(res)


def kernel(**inputs):
    in_maps = _prep_in_maps(inputs)
    try:
        per_core = _run(in_maps)
    except Exception:
        # robust fallback: the reference SPMD runner
        from concourse.bass_utils import run_bass_kernel_spmd

        nc = _get_nc()
        res = run_bass_kernel_spmd(nc, in_maps, list(range(N_CORES)))
        per_core = {
            f"out_{g}": np.stack([res.results[c][f"out_{g}"] for c in range(N_CORES)])
            for g in GROUPS
        }
    return _assemble(per_core)


def run_timed(inputs, iters=20):
    """Returns (outputs, best_wall_ns) with device-resident inputs; for test.py."""
    import time
    import jax
    from jax.sharding import NamedSharding, PartitionSpec

    in_maps = _prep_in_maps(inputs)
    sharded, in_names, out_names, out_avals, zero_outs, mesh = _get_exec()
    spec = NamedSharding(mesh, PartitionSpec("core"))
    concat_in = [
        jax.device_put(
            np.concatenate([np.asarray(in_maps[c][n]) for c in range(N_CORES)], axis=0),
            spec,
        )
        for n in in_names
    ]
    for a in concat_in:
        a.block_until_ready()

    def one_call():
        zeros = [
            np.zeros((N_CORES * z.shape[0], *z.shape[1:]), z.dtype) for z in zero_outs
        ]
        t0 = time.perf_counter_ns()
        outs = sharded(*concat_in, *zeros)
        for o in outs:
            o.block_until_ready()
        return time.perf_counter_ns() - t0, outs

    _, outs = one_call()  # warm
    best = None
    for _ in range(iters):
        dt, outs = one_call()
        best = dt if best is None else min(best, dt)
    per_core = {
        name: np.asarray(outs[i]).reshape(N_CORES, *out_avals[i].shape)
        for i, name in enumerate(out_names)
    }
    return _assemble(per_core), best
